# revision 1
# baseline (speedup 1.0000x reference)
"""BiAttention similarity kernel for Trainium2, 8-core data-parallel over batch.

Computes, per batch b:
    s0 = c @ c_weight                  # [L, 1]
    s1 = (c @ q_weight)^T              # [1, L]
    s2 = (c * cq_weight) @ q^T         # [L, L]
    s  = s0 + s1 + s2 + bias           # [L, L]

Shapes (hardcoded): B=8, L=2048, D=256, fp32 in/out.

Distribution strategy: data-parallel over batch, one batch per core. The
host-side sharding step hands each core its shard in the layout the PE
array consumes: d-major (transposed) fp16. All arithmetic — cq_weight
scaling, s0/s1 reductions, the GEMM, and the broadcast adds — runs on
device:
  - q^T scaled by cq_weight per-partition (d on partitions after transpose)
  - s0/s1 rows via skinny matmuls against c^T
  - main tiles: one PSUM accumulation group of 3 matmuls per [128, 512] tile
    (weight-stationary: each lhsT held across the 4 column tiles of a row chunk):
      K=2  : [s0_row; ones]^T @ [ones; s1_row + bias]   (adds s0[i] + s1[j] + bias)
      K=128: cT0^T @ qmodT0
      K=128: cT1^T @ qmodT1
  - PSUM->SBUF copy split between ScalarE and VectorE
  - 1 MiB contiguous output DMAs
"""

import numpy as np
from contextlib import ExitStack

import concourse.bass as bass
import concourse.tile as tile
from concourse import bacc, mybir
from concourse.bass_utils import run_bass_kernel_spmd

F32 = mybir.dt.float32
F16 = mybir.dt.float16

B = 8
L = 2048
D = 256
NK = D // 128          # 2 contraction chunks of 128
NI = L // 128          # 16 row chunks
MAIN_N = 512           # moving free dim; matmul output must stay in one PSUM bank
NJ = L // MAIN_N

# set by test harness to request an NTFF trace; results stashed in LAST_RESULTS
TRACE = False
LAST_RESULTS = None

_NC_CACHE = None


def build_body(ctx: ExitStack, tc: tile.TileContext, aps: dict):
    nc = tc.nc
    ct_d, qt_d, cw_d, qw_d, cqw_d, bias_d, s_d = (
        aps["ct"], aps["qt"], aps["c_weight"], aps["q_weight"],
        aps["cq_weight"], aps["bias"], aps["s"],
    )

    consts = ctx.enter_context(tc.tile_pool(name="consts", bufs=1))
    psum = ctx.enter_context(tc.tile_pool(name="psum", bufs=8, space="PSUM"))
    outp = ctx.enter_context(tc.tile_pool(name="outp", bufs=16))

    # ---- small constants -------------------------------------------------
    cw16 = consts.tile([128, NK], F16)
    nc.gpsimd.dma_start(cw16[:], cw_d.rearrange("(k p) one -> p (k one)", p=128))
    qw16 = consts.tile([128, NK], F16)
    nc.gpsimd.dma_start(qw16[:], qw_d.rearrange("(k p) one -> p (k one)", p=128))
    cqw32 = consts.tile([128, NK], F32)
    nc.gpsimd.dma_start(cqw32[:], cqw_d.rearrange("a b (k p) -> p (a b k)", p=128))
    bias_sb = consts.tile([1, 1], F32)
    nc.gpsimd.dma_start(bias_sb[:], bias_d[None, :])

    # transposed fp16 operands: cT_k[d, i], qmodT_k[d, j] for d-chunk k,
    # loaded in column quarters for finer-grained downstream readiness.
    cT = [consts.tile([128, L], F16, tag=f"cT{k}", name=f"cT{k}")
          for k in range(NK)]
    qT = [consts.tile([128, L], F16, tag=f"qT{k}", name=f"qT{k}")
          for k in range(NK)]
    # c^T quarters on the SP HWDGE ring (rows phase consumes c^T first and
    # gates everything downstream), q^T halves on the ACT HWDGE ring.
    for quad in range(4):
        qsl = slice(quad * 512, (quad + 1) * 512)
        for k in range(NK):
            ksl = slice(k * 128, (k + 1) * 128)
            nc.sync.dma_start(cT[k][:, qsl], ct_d[ksl, qsl])
    for half in range(2):
        hsl = slice(half * 1024, (half + 1) * 1024)
        for k in range(NK):
            ksl = slice(k * 128, (k + 1) * 128)
            nc.scalar.dma_start(qT[k][:, hsl], qt_d[ksl, hsl])
            # qmodT = qT * cq_weight (per-partition scalar after transpose)
            nc.vector.tensor_scalar_mul(qT[k][:, hsl], qT[k][:, hsl],
                                        cqw32[:, k:k + 1])

    # augmented-K rows
    ex_lhs = consts.tile([2, L], F16)   # p0 = s0 row, p1 = ones
    ex_rhs = consts.tile([2, L], F16)   # p0 = ones,   p1 = s1 row + bias
    s1_stage = consts.tile([1, L], F16)
    nc.gpsimd.memset(ex_lhs[0:2, :], 1.0)   # p0 overwritten by s0 row below
    nc.gpsimd.memset(ex_rhs[0:2, :], 1.0)   # p1 overwritten by s1 row below

    # ---- s0 / s1 rows ----------------------------------------------------
    # s0 = c @ c_weight, s1 = c @ q_weight; both as [1, L] rows via
    # out[1, N] = w_chunk[128, 1]^T @ cT_chunk[128, N], accumulated over k.
    for jj in range(4):
        sl = slice(jj * 512, (jj + 1) * 512)
        row0_ps = psum.tile([128, 512], F32, tag="main", name="row0_ps")
        row1_ps = psum.tile([128, 512], F32, tag="main", name="row1_ps")
        for k in range(NK):
            nc.tensor.matmul(row0_ps[0:1, :], cw16[:, k:k + 1], cT[k][:, sl],
                             start=(k == 0), stop=(k == NK - 1))
        for k in range(NK):
            nc.tensor.matmul(row1_ps[0:1, :], qw16[:, k:k + 1], cT[k][:, sl],
                             start=(k == 0), stop=(k == NK - 1))
        # s0 -> ex_lhs partition 0 (fp16 downcast on copy)
        nc.vector.tensor_copy(ex_lhs[0:1, sl], row0_ps[0:1, :])
        # s1 + bias -> staging row (partition 0), bounced to ex_rhs p1 by DMA
        nc.vector.tensor_scalar_add(s1_stage[0:1, sl], row1_ps[0:1, :],
                                    bias_sb[0:1, 0:1])
        nc.scalar.dma_start(ex_rhs[1:2, sl], s1_stage[0:1, sl])

    # ---- main loop: 16 row-chunks x (L/MAIN_N) column tiles --------------
    Copy = mybir.ActivationFunctionType.Copy
    for i in range(NI):
        isl = slice(i * 128, (i + 1) * 128)
        out_sb = outp.tile([128, L], F32, tag="out", name="out_sb")
        # weight-stationary: hold each lhsT across all NJ column tiles so its
        # LDWEIGHTS is paid once per sweep instead of once per matmul
        pss = [psum.tile([128, MAIN_N], F32, tag="main", name=f"ps{jj}")
               for jj in range(NJ)]
        for jj in range(NJ):
            nc.tensor.matmul(pss[jj][:], ex_lhs[:, isl],
                             ex_rhs[:, jj * MAIN_N:(jj + 1) * MAIN_N],
                             start=True, stop=False)
        for jj in range(NJ):
            nc.tensor.matmul(pss[jj][:], cT[0][:, isl],
                             qT[0][:, jj * MAIN_N:(jj + 1) * MAIN_N],
                             start=False, stop=False)
        for jj in range(NJ):
            nc.tensor.matmul(pss[jj][:], cT[1][:, isl],
                             qT[1][:, jj * MAIN_N:(jj + 1) * MAIN_N],
                             start=False, stop=True)
            # split the PSUM->SBUF copy between ScalarE and VectorE
            sl = slice(jj * MAIN_N, (jj + 1) * MAIN_N)
            if jj % 2 == 0:
                nc.scalar.activation(out_sb[:, sl], pss[jj][:], Copy)
            else:
                nc.vector.tensor_copy(out_sb[:, sl], pss[jj][:])
        # Sync issues both output halves (its waits are cheap; keeps ACT free)
        nc.sync.dma_start(s_d[isl, 0:1024], out_sb[:, 0:1024])
        nc.sync.dma_start(s_d[isl, 1024:2048], out_sb[:, 1024:2048])


def build_nc():
    nc = bacc.Bacc("TRN2", target_bir_lowering=False, debug=False)
    aps = {
        "ct": nc.dram_tensor("ct", [D, L], F16, kind="ExternalInput").ap(),
        "qt": nc.dram_tensor("qt", [D, L], F16, kind="ExternalInput").ap(),
        "c_weight": nc.dram_tensor("c_weight", [D, 1], F32,
                                   kind="ExternalInput").ap(),
        "q_weight": nc.dram_tensor("q_weight", [D, 1], F32,
                                   kind="ExternalInput").ap(),
        "cq_weight": nc.dram_tensor("cq_weight", [1, 1, D], F32,
                                    kind="ExternalInput").ap(),
        "bias": nc.dram_tensor("bias", [1], F32, kind="ExternalInput").ap(),
        "s": nc.dram_tensor("s", [L, L], F32, kind="ExternalOutput").ap(),
    }
    with tile.TileContext(nc) as tc:
        with ExitStack() as ctx:
            build_body(ctx, tc, aps)
    nc.compile()
    return nc


def get_nc():
    global _NC_CACHE
    if _NC_CACHE is None:
        _NC_CACHE = build_nc()
    return _NC_CACHE


def kernel(c, q, c_weight, q_weight, cq_weight, bias):
    global LAST_RESULTS
    nc = get_nc()
    c = np.asarray(c, dtype=np.float32)
    q = np.asarray(q, dtype=np.float32)
    cw = np.asarray(c_weight, dtype=np.float32)
    qw = np.asarray(q_weight, dtype=np.float32)
    cqw = np.asarray(cq_weight, dtype=np.float32)
    bias = np.asarray(bias, dtype=np.float32)
    # shard: batch b -> core b, shards laid out d-major (transposed) fp16
    in_maps = [
        {
            "ct": np.ascontiguousarray(c[b].T).astype(np.float16),
            "qt": np.ascontiguousarray(q[b].T).astype(np.float16),
            "c_weight": cw,
            "q_weight": qw,
            "cq_weight": cqw,
            "bias": bias,
        }
        for b in range(B)
    ]
    res = run_bass_kernel_spmd(nc, in_maps, core_ids=list(range(B)), trace=TRACE)
    LAST_RESULTS = res
    return np.stack([res.results[b]["s"] for b in range(B)], axis=0)



# revision 2
# speedup vs baseline: 1.0879x; 1.0879x over previous
"""BiAttention similarity kernel for Trainium2, 8-core data-parallel over batch.

Computes, per batch b:
    s0 = c @ c_weight                  # [L, 1]
    s1 = (c @ q_weight)^T              # [1, L]
    s2 = (c * cq_weight) @ q^T         # [L, L]
    s  = s0 + s1 + s2 + bias           # [L, L]

Shapes (hardcoded): B=8, L=2048, D=256, fp32 in/out.

Distribution: data-parallel over batch, one batch per core. Host hands each
core its shard d-major (transposed) fp16; the device returns s in fp16
(quantization ~5e-4 rel, well under tolerance) and the host upcasts to fp32.
Halving the output bytes halves the dominant HBM write traffic.

Device dataflow per core:
  - qmod = q^T * cq_weight (per-partition scalar after transpose), on DVE.
  - s1 row via skinny matmuls against c^T; +bias and fp16 cast on ACT.
  - s1b = broadcast of (s1+bias) to all 128 partitions via a K=1 ones matmul,
    cast to fp16 by ACT: [128, 2048] in SBUF.
  - main loop over 16 row chunks: PE fills two [128,1024] PSUM tiles
    (4 matmuls of N=512 per K-chunk, K=128 x 2), plus a tiny N=1 matmul per
    K-chunk computing s0 for the chunk into a persistent PSUM column
    (rides the same LDWEIGHTS as the main matmuls).
  - drain: DVE scalar_tensor_tensor fuses (psumA + s0[i]) + s1b -> fp16 out
    for the first half; ACT does a plain fp32->fp16 copy of the second half
    and DVE adds (s0[i] + s1b) in a 16-bit in-place pass.
  - one contiguous 512 KiB output DMA per chunk on the sync HWDGE ring.
"""

import numpy as np
from contextlib import ExitStack

import concourse.bass as bass
import concourse.tile as tile
from concourse import bacc, mybir
from concourse.bass_utils import run_bass_kernel_spmd

F32 = mybir.dt.float32
F16 = mybir.dt.float16
ADD = mybir.AluOpType.add

B = 8
L = 2048
D = 256
NK = D // 128          # 2 contraction chunks of 128
NI = L // 128          # 16 row chunks
HALF = 1024            # column split: A = [0:1024] (DVE), B = [1024:2048] (ACT)

# set by test harness to request an NTFF trace; results stashed in LAST_RESULTS
TRACE = False
LAST_RESULTS = None

_NC_CACHE = None


def build_body(ctx: ExitStack, tc: tile.TileContext, aps: dict):
    nc = tc.nc
    ct_d, qt_d, cw_d, qw_d, cqw_d, bias_d, s_d = (
        aps["ct"], aps["qt"], aps["c_weight"], aps["q_weight"],
        aps["cq_weight"], aps["bias"], aps["s"],
    )
    Copy = mybir.ActivationFunctionType.Copy

    consts = ctx.enter_context(tc.tile_pool(name="consts", bufs=1))
    psum = ctx.enter_context(tc.tile_pool(name="psum", bufs=3, space="PSUM"))
    psum1 = ctx.enter_context(tc.tile_pool(name="psum1", bufs=1, space="PSUM"))
    outp = ctx.enter_context(tc.tile_pool(name="outp", bufs=6))

    # ---- small constants -------------------------------------------------
    cw16 = consts.tile([128, NK], F16)
    nc.gpsimd.dma_start(cw16[:], cw_d.rearrange("(k p) one -> p (k one)", p=128))
    qw16 = consts.tile([128, NK], F16)
    nc.gpsimd.dma_start(qw16[:], qw_d.rearrange("(k p) one -> p (k one)", p=128))
    cqw32 = consts.tile([128, NK], F32)
    nc.gpsimd.dma_start(cqw32[:], cqw_d.rearrange("a b (k p) -> p (a b k)", p=128))
    bias_sb = consts.tile([1, 1], F32)
    nc.gpsimd.dma_start(bias_sb[:], bias_d[None, :])
    ones16 = consts.tile([1, 128], F16)
    nc.gpsimd.memset(ones16[0:1, :], 1.0)

    # transposed fp16 operands: cT_k[d, i], qT_k[d, j] for d-chunk k.
    cT = [consts.tile([128, L], F16, tag=f"cT{k}", name=f"cT{k}")
          for k in range(NK)]
    qT = [consts.tile([128, L], F16, tag=f"qT{k}", name=f"qT{k}")
          for k in range(NK)]
    # c^T quarters on the SP HWDGE ring (rows phase consumes c^T first and
    # gates everything downstream), q^T halves on the ACT HWDGE ring.
    for quad in range(4):
        qsl = slice(quad * 512, (quad + 1) * 512)
        for k in range(NK):
            ksl = slice(k * 128, (k + 1) * 128)
            nc.sync.dma_start(cT[k][:, qsl], ct_d[ksl, qsl])
    for half in range(2):
        hsl = slice(half * 1024, (half + 1) * 1024)
        for k in range(NK):
            ksl = slice(k * 128, (k + 1) * 128)
            nc.scalar.dma_start(qT[k][:, hsl], qt_d[ksl, hsl])
            # qmod = qT * cq_weight (per-partition scalar after transpose)
            nc.vector.tensor_scalar_mul(qT[k][:, hsl], qT[k][:, hsl],
                                        cqw32[:, k:k + 1])

    # ---- s1 row + broadcast ---------------------------------------------
    # s1 = c @ q_weight as [1, L] via out[1, N] = qw_chunk[128,1]^T @ cT[128,N]
    s1p16 = consts.tile([1, L], F16)     # s1 + bias, fp16
    s1b16 = consts.tile([128, L], F16)   # broadcast to all partitions
    rows = [psum.tile([128, HALF], F32, tag="main", name=f"rows{h}")
            for h in range(2)]
    for jj in range(4):
        rp = rows[jj // 2]
        csl = slice((jj % 2) * 512, (jj % 2) * 512 + 512)
        sl = slice(jj * 512, (jj + 1) * 512)
        for k in range(NK):
            nc.tensor.matmul(rp[0:1, csl], qw16[:, k:k + 1], cT[k][:, sl],
                             start=(k == 0), stop=(k == NK - 1))
        # s1 + bias -> fp16 staging row (ACT, Identity supports AP bias)
        nc.scalar.add(s1p16[0:1, sl], rp[0:1, csl], bias_sb[0:1, 0:1])
    # broadcast via K=1 matmul: ones[1,128]^T @ s1p16[1,N] -> [128, N]
    bc = [psum.tile([128, HALF], F32, tag="main", name=f"bc{h}")
          for h in range(2)]
    for jj in range(4):
        nc.tensor.matmul(bc[jj // 2][:, (jj % 2) * 512:(jj % 2) * 512 + 512],
                         ones16[0:1, :], s1p16[0:1, jj * 512:(jj + 1) * 512],
                         start=True, stop=True)
    for h in range(2):
        nc.scalar.activation(s1b16[:, h * HALF:(h + 1) * HALF], bc[h][:], Copy)

    # ---- main loop: 16 row chunks ----------------------------------------
    # persistent s0 column store: PSUM accumulator + SBUF staging
    s0c_ps = psum1.tile([128, NI], F32, tag="s0c", name="s0c_ps")
    s0_sb = consts.tile([128, NI], F32)

    for i in range(NI):
        isl = slice(i * 128, (i + 1) * 128)
        out_sb = outp.tile([128, L], F16, tag="out", name="out_sb")
        pa = psum.tile([128, HALF], F32, tag="main", name="pa")
        pb = psum.tile([128, HALF], F32, tag="main", name="pb")
        for k in range(NK):
            first, last = (k == 0), (k == NK - 1)
            # tiny s0 matmul rides the same LDWEIGHTS as the main matmuls
            nc.tensor.matmul(s0c_ps[:, i:i + 1], cT[k][:, isl],
                             cw16[:, k:k + 1], start=first, stop=last)
            for jj in range(4):
                ps = pa if jj < 2 else pb
                csl = slice((jj % 2) * 512, (jj % 2) * 512 + 512)
                nc.tensor.matmul(ps[:, csl], cT[k][:, isl],
                                 qT[k][:, jj * 512:(jj + 1) * 512],
                                 start=first, stop=last)
        # s0 column for this chunk -> SBUF (tiny DVE copy)
        nc.vector.tensor_copy(s0_sb[:, i:i + 1], s0c_ps[:, i:i + 1])
        # A half: fused (psum + s0) + s1b -> fp16, on DVE
        nc.vector.scalar_tensor_tensor(
            out_sb[:, 0:HALF], pa[:], s0_sb[:, i:i + 1], s1b16[:, 0:HALF],
            ADD, ADD)
        # B half: plain fp32->fp16 copy on ACT, then 16-bit fused add on DVE
        nc.scalar.activation(out_sb[:, HALF:L], pb[:], Copy)
        nc.vector.scalar_tensor_tensor(
            out_sb[:, HALF:L], out_sb[:, HALF:L], s0_sb[:, i:i + 1],
            s1b16[:, HALF:L], ADD, ADD)
        # one contiguous 512 KiB output DMA per chunk
        nc.sync.dma_start(s_d[isl, :], out_sb[:, :])


def build_nc():
    nc = bacc.Bacc("TRN2", target_bir_lowering=False, debug=False)
    aps = {
        "ct": nc.dram_tensor("ct", [D, L], F16, kind="ExternalInput").ap(),
        "qt": nc.dram_tensor("qt", [D, L], F16, kind="ExternalInput").ap(),
        "c_weight": nc.dram_tensor("c_weight", [D, 1], F32,
                                   kind="ExternalInput").ap(),
        "q_weight": nc.dram_tensor("q_weight", [D, 1], F32,
                                   kind="ExternalInput").ap(),
        "cq_weight": nc.dram_tensor("cq_weight", [1, 1, D], F32,
                                    kind="ExternalInput").ap(),
        "bias": nc.dram_tensor("bias", [1], F32, kind="ExternalInput").ap(),
        "s": nc.dram_tensor("s", [L, L], F16, kind="ExternalOutput").ap(),
    }
    with tile.TileContext(nc) as tc:
        with ExitStack() as ctx:
            build_body(ctx, tc, aps)
    nc.compile()
    return nc


def get_nc():
    global _NC_CACHE
    if _NC_CACHE is None:
        _NC_CACHE = build_nc()
    return _NC_CACHE


def kernel(c, q, c_weight, q_weight, cq_weight, bias):
    global LAST_RESULTS
    nc = get_nc()
    c = np.asarray(c, dtype=np.float32)
    q = np.asarray(q, dtype=np.float32)
    cw = np.asarray(c_weight, dtype=np.float32)
    qw = np.asarray(q_weight, dtype=np.float32)
    cqw = np.asarray(cq_weight, dtype=np.float32)
    bias = np.asarray(bias, dtype=np.float32)
    # shard: batch b -> core b, shards laid out d-major (transposed) fp16
    in_maps = [
        {
            "ct": np.ascontiguousarray(c[b].T).astype(np.float16),
            "qt": np.ascontiguousarray(q[b].T).astype(np.float16),
            "c_weight": cw,
            "q_weight": qw,
            "cq_weight": cqw,
            "bias": bias,
        }
        for b in range(B)
    ]
    res = run_bass_kernel_spmd(nc, in_maps, core_ids=list(range(B)), trace=TRACE)
    LAST_RESULTS = res
    return np.stack([res.results[b]["s"].astype(np.float32) for b in range(B)],
                    axis=0)


# revision 4
# speedup vs baseline: 1.2361x; 1.1362x over previous
"""BiAttention similarity kernel for Trainium2, 8-core data-parallel over batch.

Computes, per batch b:
    s0 = c @ c_weight                  # [L, 1]
    s1 = (c @ q_weight)^T              # [1, L]
    s2 = (c * cq_weight) @ q^T         # [L, L]
    s  = s0 + s1 + s2 + bias           # [L, L]

Shapes (hardcoded): B=8, L=2048, D=256, fp32 in/out.

Distribution: data-parallel over batch, one batch per core. Host hands each
core its shard d-major (transposed) fp16; the device returns s in fp16
(quantization ~5e-4 rel, well under tolerance) and the host upcasts to fp32.
Halving the output bytes halves the dominant HBM write traffic.

Device dataflow per core:
  - qmod = q^T * cq_weight (per-partition scalar after transpose), on DVE.
  - s1 row via skinny matmuls against c^T; +bias and fp16 cast on ACT.
  - s1b = broadcast of (s1+bias) to all 128 partitions via a K=1 ones matmul,
    cast to fp16 by ACT: [128, 2048] in SBUF.
  - main loop over 16 row chunks: PE fills two [128,1024] PSUM tiles
    (4 matmuls of N=512 per K-chunk, K=128 x 2), plus a tiny N=1 matmul per
    K-chunk computing s0 for the chunk into a persistent PSUM column
    (rides the same LDWEIGHTS as the main matmuls).
  - drain: DVE scalar_tensor_tensor fuses (psumA + s0[i]) + s1b -> fp16 out
    for the first half; ACT does a plain fp32->fp16 copy of the second half
    and DVE adds (s0[i] + s1b) in a 16-bit in-place pass.
  - one contiguous 512 KiB output DMA per chunk on the sync HWDGE ring.
"""

import numpy as np
from contextlib import ExitStack

import concourse.bass as bass
import concourse.tile as tile
from concourse import bacc, mybir
from concourse.bass_utils import run_bass_kernel_spmd

F32 = mybir.dt.float32
F16 = mybir.dt.float16
ADD = mybir.AluOpType.add

B = 8
L = 2048
D = 256
NK = D // 128          # 2 contraction chunks of 128
NI = L // 128          # 16 row chunks
HALF = 1024            # column split: A = [0:1024] (DVE), B = [1024:2048] (ACT)

# set by test harness to request an NTFF trace; results stashed in LAST_RESULTS
TRACE = False
LAST_RESULTS = None

_NC_CACHE = None


def build_body(ctx: ExitStack, tc: tile.TileContext, aps: dict):
    nc = tc.nc
    ct_d, qt_d, cw_d, qw_d, cqw_d, bias_d, s_d = (
        aps["ct"], aps["qt"], aps["c_weight"], aps["q_weight"],
        aps["cq_weight"], aps["bias"], aps["s"],
    )
    Copy = mybir.ActivationFunctionType.Copy

    consts = ctx.enter_context(tc.tile_pool(name="consts", bufs=1))
    psum = ctx.enter_context(tc.tile_pool(name="psum", bufs=3, space="PSUM"))
    psum1 = ctx.enter_context(tc.tile_pool(name="psum1", bufs=1, space="PSUM"))
    outp = ctx.enter_context(tc.tile_pool(name="outp", bufs=6))

    # ---- small constants -------------------------------------------------
    cw16 = consts.tile([128, NK], F16)
    nc.gpsimd.dma_start(cw16[:], cw_d.rearrange("(k p) one -> p (k one)", p=128))
    qw16 = consts.tile([128, NK], F16)
    nc.gpsimd.dma_start(qw16[:], qw_d.rearrange("(k p) one -> p (k one)", p=128))
    cqw32 = consts.tile([128, NK], F32)
    nc.gpsimd.dma_start(cqw32[:], cqw_d.rearrange("a b (k p) -> p (a b k)", p=128))
    bias_sb = consts.tile([1, 1], F32)
    nc.gpsimd.dma_start(bias_sb[:], bias_d[None, :])
    ones16 = consts.tile([1, 128], F16)
    nc.gpsimd.memset(ones16[0:1, :], 1.0)

    # transposed fp16 operands: cT_k[d, i], qT_k[d, j] for d-chunk k.
    cT = [consts.tile([128, L], F16, tag=f"cT{k}", name=f"cT{k}")
          for k in range(NK)]
    qT = [consts.tile([128, L], F16, tag=f"qT{k}", name=f"qT{k}")
          for k in range(NK)]
    # one 512 KiB contiguous DMA per k-half per ring: c^T on the SP HWDGE
    # ring, q^T on the ACT HWDGE ring.
    for k in range(NK):
        ksl = slice(k * 128, (k + 1) * 128)
        nc.sync.dma_start(cT[k][:, :], ct_d[ksl, :])
        nc.scalar.dma_start(qT[k][:, :], qt_d[ksl, :])
        # qmod = qT * cq_weight (per-partition scalar after transpose)
        nc.vector.tensor_scalar_mul(qT[k][:, :], qT[k][:, :],
                                    cqw32[:, k:k + 1])

    # ---- s1 row + broadcast ---------------------------------------------
    # s1 = c @ q_weight as [1, L] via out[1, N] = qw_chunk[128,1]^T @ cT[128,N]
    s1p16 = consts.tile([1, L], F16)     # s1 + bias, fp16
    s1b16 = consts.tile([128, L], F16)   # broadcast to all partitions
    rows = [psum.tile([128, HALF], F32, tag="main", name=f"rows{h}")
            for h in range(2)]
    for k in range(NK):
        for jj in range(4):
            rp = rows[jj // 2]
            csl = slice((jj % 2) * 512, (jj % 2) * 512 + 512)
            sl = slice(jj * 512, (jj + 1) * 512)
            nc.tensor.matmul(rp[0:1, csl], qw16[:, k:k + 1], cT[k][:, sl],
                             start=(k == 0), stop=(k == NK - 1))
    for jj in range(4):
        sl = slice(jj * 512, (jj + 1) * 512)
        csl = slice((jj % 2) * 512, (jj % 2) * 512 + 512)
        # s1 + bias -> fp16 staging row (ACT, Identity supports AP bias)
        nc.scalar.add(s1p16[0:1, sl], rows[jj // 2][0:1, csl],
                      bias_sb[0:1, 0:1])
    # broadcast via K=1 matmul: ones[1,128]^T @ s1p16[1,N] -> [128, N]
    bc = [psum.tile([128, HALF], F32, tag="main", name=f"bc{h}")
          for h in range(2)]
    for jj in range(4):
        nc.tensor.matmul(bc[jj // 2][:, (jj % 2) * 512:(jj % 2) * 512 + 512],
                         ones16[0:1, :], s1p16[0:1, jj * 512:(jj + 1) * 512],
                         start=True, stop=True)
    # split the broadcast copies: DVE makes the A half (needed by the fused
    # drain first), ACT the B half; they run concurrently.
    nc.vector.tensor_copy(s1b16[:, 0:HALF], bc[0][:])
    nc.scalar.activation(s1b16[:, HALF:L], bc[1][:], Copy)

    # ---- main loop: 16 row chunks ----------------------------------------
    # persistent s0 column store: PSUM accumulator + SBUF staging
    s0c_ps = psum1.tile([128, NI], F32, tag="s0c", name="s0c_ps")
    s0_sb = consts.tile([128, NI], F32)

    for i in range(NI):
        isl = slice(i * 128, (i + 1) * 128)
        out_sb = outp.tile([128, L], F16, tag="out", name="out_sb")
        pa = psum.tile([128, HALF], F32, tag="main", name="pa")
        pb = psum.tile([128, HALF], F32, tag="main", name="pb")
        for k in range(NK):
            first, last = (k == 0), (k == NK - 1)
            # tiny s0 matmul rides the same LDWEIGHTS as the main matmuls
            nc.tensor.matmul(s0c_ps[:, i:i + 1], cT[k][:, isl],
                             cw16[:, k:k + 1], start=first, stop=last)
            for jj in range(4):
                ps = pa if jj < 2 else pb
                csl = slice((jj % 2) * 512, (jj % 2) * 512 + 512)
                nc.tensor.matmul(ps[:, csl], cT[k][:, isl],
                                 qT[k][:, jj * 512:(jj + 1) * 512],
                                 start=first, stop=last)
        # s0 column for this chunk -> SBUF (tiny ACT copy; keeps DVE free)
        nc.scalar.copy(s0_sb[:, i:i + 1], s0c_ps[:, i:i + 1])
        # A half: fused (psum + s0) + s1b -> fp16, on DVE
        nc.vector.scalar_tensor_tensor(
            out_sb[:, 0:HALF], pa[:], s0_sb[:, i:i + 1], s1b16[:, 0:HALF],
            ADD, ADD)
        # B half: ACT folds the s0 add into the fp32->fp16 copy (Identity
        # with per-partition AP bias), then DVE adds s1b in an all-fp16
        # in-place pass (2x DVE mode eligible)
        nc.scalar.add(out_sb[:, HALF:L], pb[:], s0_sb[:, i:i + 1])
        nc.vector.tensor_add(out_sb[:, HALF:L], out_sb[:, HALF:L],
                             s1b16[:, HALF:L])
        # one contiguous 512 KiB output DMA per chunk
        nc.sync.dma_start(s_d[isl, :], out_sb[:, :])


def build_nc():
    nc = bacc.Bacc("TRN2", target_bir_lowering=False, debug=False)
    aps = {
        "ct": nc.dram_tensor("ct", [D, L], F16, kind="ExternalInput").ap(),
        "qt": nc.dram_tensor("qt", [D, L], F16, kind="ExternalInput").ap(),
        "c_weight": nc.dram_tensor("c_weight", [D, 1], F32,
                                   kind="ExternalInput").ap(),
        "q_weight": nc.dram_tensor("q_weight", [D, 1], F32,
                                   kind="ExternalInput").ap(),
        "cq_weight": nc.dram_tensor("cq_weight", [1, 1, D], F32,
                                    kind="ExternalInput").ap(),
        "bias": nc.dram_tensor("bias", [1], F32, kind="ExternalInput").ap(),
        "s": nc.dram_tensor("s", [L, L], F16, kind="ExternalOutput").ap(),
    }
    with tile.TileContext(nc) as tc:
        with ExitStack() as ctx:
            build_body(ctx, tc, aps)
    nc.compile()
    return nc


def get_nc():
    global _NC_CACHE
    if _NC_CACHE is None:
        _NC_CACHE = build_nc()
    return _NC_CACHE


def kernel(c, q, c_weight, q_weight, cq_weight, bias):
    global LAST_RESULTS
    nc = get_nc()
    c = np.asarray(c, dtype=np.float32)
    q = np.asarray(q, dtype=np.float32)
    cw = np.asarray(c_weight, dtype=np.float32)
    qw = np.asarray(q_weight, dtype=np.float32)
    cqw = np.asarray(cq_weight, dtype=np.float32)
    bias = np.asarray(bias, dtype=np.float32)
    # shard: batch b -> core b, shards laid out d-major (transposed) fp16
    in_maps = [
        {
            "ct": np.ascontiguousarray(c[b].T).astype(np.float16),
            "qt": np.ascontiguousarray(q[b].T).astype(np.float16),
            "c_weight": cw,
            "q_weight": qw,
            "cq_weight": cqw,
            "bias": bias,
        }
        for b in range(B)
    ]
    res = run_bass_kernel_spmd(nc, in_maps, core_ids=list(range(B)), trace=TRACE)
    LAST_RESULTS = res
    return np.stack([res.results[b]["s"].astype(np.float32) for b in range(B)],
                    axis=0)


# revision 7
# speedup vs baseline: 1.2384x; 1.0019x over previous
"""BiAttention similarity kernel for Trainium2, 8-core data-parallel over batch.

Computes, per batch b:
    s0 = c @ c_weight                  # [L, 1]
    s1 = (c @ q_weight)^T              # [1, L]
    s2 = (c * cq_weight) @ q^T         # [L, L]
    s  = s0 + s1 + s2 + bias           # [L, L]

Shapes (hardcoded): B=8, L=2048, D=256, fp32 in/out.

Distribution: data-parallel over batch, one batch per core. Host hands each
core its shard d-major (transposed) fp16 plus a packed per-partition weight
tile; the device returns s in fp16 (quantization ~5e-4 rel, well under
tolerance) and the host upcasts to fp32 — halving the dominant HBM write.

Device dataflow per core:
  - warmup: a few dummy matmuls while inputs stream in, so the PE HAM clock
    gate reaches 8/8 before real work starts.
  - inputs interleaved across both HWDGE rings (ct halves first, qt second)
    so c^T completes early for the s1-row phase.
  - qmod = q^T * cq_weight: k0 half on ACT (scale AP), k1 half on DVE.
  - s1 row via skinny matmuls; single ACT pass adds bias + casts fp16;
    broadcast to 128 partitions via a K=1 ones matmul; fp16 copies split
    DVE (A cols) / ACT (B cols).
  - main loop over 16 row chunks: PE fills one [128,512] A tile + one
    [128,1536] B tile per chunk (2 K-chunks of 128), plus a tiny N=1 matmul
    per K-chunk computing s0 into a persistent PSUM column (~26 ns of issue
    each, rides the same LDWEIGHTS).
  - drains: DVE scalar_tensor_tensor fuses (psumA + s0) + s1b -> fp16;
    ACT folds the s0 add into its fp32->fp16 copy of B (Identity + AP
    bias); DVE adds s1b over B in an all-fp16 2x-mode tensor_tensor.
  - one contiguous 512 KiB output DMA per chunk on the sync ring; the last
    chunk is drained B-first with split DMAs to shorten the tail.
"""

import numpy as np
from contextlib import ExitStack

import concourse.bass as bass
import concourse.tile as tile
from concourse import bacc, mybir
from concourse.bass_utils import run_bass_kernel_spmd

F32 = mybir.dt.float32
F16 = mybir.dt.float16
ADD = mybir.AluOpType.add

B = 8
L = 2048
D = 256
NK = D // 128          # 2 contraction chunks of 128
NI = L // 128          # 16 row chunks
ASPLIT = 512           # A = [0:512] (DVE fused drain), B = [512:2048] (ACT)
N_WARMUP = 10          # dummy matmuls to warm the PE clock gate

TRACE = False
LAST_RESULTS = None

_NC_CACHE = None


def build_body(ctx: ExitStack, tc: tile.TileContext, aps: dict):
    nc = tc.nc
    ct_d, qt_d, w_d, s_d = aps["ct"], aps["qt"], aps["wconsts"], aps["s"]
    Copy = mybir.ActivationFunctionType.Copy

    consts = ctx.enter_context(tc.tile_pool(name="consts", bufs=1))
    psA = ctx.enter_context(tc.tile_pool(name="psA", bufs=1, space="PSUM"))
    psB = ctx.enter_context(tc.tile_pool(name="psB", bufs=2, space="PSUM"))
    ps0 = ctx.enter_context(tc.tile_pool(name="ps0", bufs=1, space="PSUM"))
    outp = ctx.enter_context(tc.tile_pool(name="outp", bufs=6))

    # ---- constants -------------------------------------------------------
    # host-packed [128, 7] fp32: cols 0-1 cw(k0,k1), 2-3 qw, 4-5 cqw, 6 bias
    # (single DMA; per-partition layout avoids a 2-byte-packet scatter spray)
    wc = consts.tile([128, 7], F32)
    nc.gpsimd.dma_start(wc[:], w_d)
    cw16 = consts.tile([128, NK], F16)
    nc.vector.tensor_copy(cw16[:], wc[:, 0:2])
    qw16 = consts.tile([128, NK], F16)
    nc.vector.tensor_copy(qw16[:], wc[:, 2:4])
    ones16 = consts.tile([1, 128], F16)
    nc.gpsimd.memset(ones16[0:1, :], 1.0)
    dummy16 = consts.tile([1, 512], F16)
    nc.gpsimd.memset(dummy16[0:1, :], 0.0)

    # ---- PE warmup while inputs stream ----------------------------------
    warm = psA.tile([128, ASPLIT], F32, tag="A", name="warm")
    for w in range(N_WARMUP):
        nc.tensor.matmul(warm[:], ones16[0:1, :], dummy16[0:1, :],
                         start=True, stop=True)

    # ---- inputs: ct halves first (gates the s1 row phase), qt second -----
    cT = [consts.tile([128, L], F16, tag=f"cT{k}", name=f"cT{k}")
          for k in range(NK)]
    qT = [consts.tile([128, L], F16, tag=f"qT{k}", name=f"qT{k}")
          for k in range(NK)]
    nc.sync.dma_start(cT[0][:, :], ct_d[0:128, :])
    nc.scalar.dma_start(cT[1][:, :], ct_d[128:256, :])
    nc.sync.dma_start(qT[0][:, :], qt_d[0:128, :])
    nc.scalar.dma_start(qT[1][:, :], qt_d[128:256, :])
    # qmod = qT * cq_weight (per-partition scalar): k0 on ACT, k1 on DVE
    nc.scalar.mul(qT[0][:, :], qT[0][:, :], wc[:, 4:5])
    nc.vector.tensor_scalar_mul(qT[1][:, :], qT[1][:, :], wc[:, 5:6])

    # ---- s1 row + broadcast ---------------------------------------------
    s1p16 = consts.tile([1, L], F16)     # s1 + bias, fp16
    s1b16 = consts.tile([128, L], F16)   # broadcast to all partitions
    rows = [psB.tile([128, 1024], F32, tag="B", name=f"rows{h}",
                     padded_shape=[128, 1536])
            for h in range(2)]
    for k in range(NK):
        for jj in range(4):
            rp = rows[jj // 2]
            csl = slice((jj % 2) * 512, (jj % 2) * 512 + 512)
            sl = slice(jj * 512, (jj + 1) * 512)
            nc.tensor.matmul(rp[0:1, csl], qw16[:, k:k + 1], cT[k][:, sl],
                             start=(k == 0), stop=(k == NK - 1))
    # s1 + bias -> fp16 staging row (ACT Identity supports AP bias)
    for h in range(2):
        nc.scalar.add(s1p16[0:1, h * 1024:(h + 1) * 1024],
                      rows[h][0:1, 0:1024], wc[0:1, 6:7])
    # broadcast via K=1 matmul: ones[1,128]^T @ s1p16[1,N] -> [128, N]
    bc = [psB.tile([128, 1024], F32, tag="B", name=f"bc{h}",
                   padded_shape=[128, 1536])
          for h in range(2)]
    for jj in range(4):
        nc.tensor.matmul(bc[jj // 2][:, (jj % 2) * 512:(jj % 2) * 512 + 512],
                         ones16[0:1, :], s1p16[0:1, jj * 512:(jj + 1) * 512],
                         start=True, stop=True)
    # fp16 broadcast copies: DVE makes the A-side cols, ACT the rest
    nc.vector.tensor_copy(s1b16[:, 0:1024], bc[0][:, 0:1024])
    nc.scalar.activation(s1b16[:, 1024:L], bc[1][:, 0:1024], Copy)

    # ---- main loop: 16 row chunks ----------------------------------------
    s0c_ps = ps0.tile([128, NI], F32, tag="s0c", name="s0c_ps")
    s0_sb = consts.tile([128, NI], F32)

    for i in range(NI):
        isl = slice(i * 128, (i + 1) * 128)
        last_chunk = (i == NI - 1)
        out_sb = outp.tile([128, L], F16, tag="out", name="out_sb")
        pa = psA.tile([128, ASPLIT], F32, tag="A", name="pa")
        pb = psB.tile([128, L - ASPLIT], F32, tag="B", name="pb")
        for k in range(NK):
            first, last = (k == 0), (k == NK - 1)
            # tiny s0 matmul rides the same LDWEIGHTS as the main matmuls
            nc.tensor.matmul(s0c_ps[:, i:i + 1], cT[k][:, isl],
                             cw16[:, k:k + 1], start=first, stop=last)
            mms = [(pb, jj) for jj in range(3)] + [(pa, None)]
            if last_chunk:
                mms = mms  # B tiles first, A last: shortens the drain tail
            else:
                mms = [(pa, None)] + [(pb, jj) for jj in range(3)]
            for ps, jj in mms:
                if jj is None:
                    nc.tensor.matmul(pa[:], cT[k][:, isl],
                                     qT[k][:, 0:ASPLIT],
                                     start=first, stop=last)
                else:
                    nc.tensor.matmul(pb[:, jj * 512:(jj + 1) * 512],
                                     cT[k][:, isl],
                                     qT[k][:, ASPLIT + jj * 512:
                                            ASPLIT + (jj + 1) * 512],
                                     start=first, stop=last)
        # s0 column for this chunk -> SBUF (tiny DVE copy; ACT bias needs SBUF)
        nc.vector.tensor_copy(s0_sb[:, i:i + 1], s0c_ps[:, i:i + 1])

        def drain_a():
            # A: one fused DVE op
            nc.vector.scalar_tensor_tensor(
                out_sb[:, 0:ASPLIT], pa[:], s0_sb[:, i:i + 1],
                s1b16[:, 0:ASPLIT], ADD, ADD)

        def drain_b():
            # B: ACT folds the s0 add into the fp32->fp16 copy, DVE adds
            # s1b in an all-fp16 2x-mode pass
            nc.scalar.add(out_sb[:, ASPLIT:L], pb[:], s0_sb[:, i:i + 1])
            nc.vector.tensor_add(out_sb[:, ASPLIT:L], out_sb[:, ASPLIT:L],
                                 s1b16[:, ASPLIT:L])

        if last_chunk:
            # B filled first: drain + ship it while A finishes
            drain_b()
            nc.sync.dma_start(s_d[isl, ASPLIT:L], out_sb[:, ASPLIT:L])
            drain_a()
            nc.sync.dma_start(s_d[isl, 0:ASPLIT], out_sb[:, 0:ASPLIT])
        else:
            # stt32 first on the DVE FIFO so the single-buffered A tile
            # frees before the next chunk's fill needs it
            drain_a()
            drain_b()
            nc.sync.dma_start(s_d[isl, :], out_sb[:, :])


def build_nc():
    nc = bacc.Bacc("TRN2", target_bir_lowering=False, debug=False)
    aps = {
        "ct": nc.dram_tensor("ct", [D, L], F16, kind="ExternalInput").ap(),
        "qt": nc.dram_tensor("qt", [D, L], F16, kind="ExternalInput").ap(),
        "wconsts": nc.dram_tensor("wconsts", [128, 7], F32,
                                  kind="ExternalInput").ap(),
        "s": nc.dram_tensor("s", [L, L], F16, kind="ExternalOutput").ap(),
    }
    with tile.TileContext(nc) as tc:
        with ExitStack() as ctx:
            build_body(ctx, tc, aps)
    nc.compile()
    return nc


def get_nc():
    global _NC_CACHE
    if _NC_CACHE is None:
        _NC_CACHE = build_nc()
    return _NC_CACHE


def kernel(c, q, c_weight, q_weight, cq_weight, bias):
    global LAST_RESULTS
    nc = get_nc()
    c = np.asarray(c, dtype=np.float32)
    q = np.asarray(q, dtype=np.float32)
    cw = np.asarray(c_weight, dtype=np.float32).reshape(2, 128).T  # [128, 2]
    qw = np.asarray(q_weight, dtype=np.float32).reshape(2, 128).T
    cqw = np.asarray(cq_weight, dtype=np.float32).reshape(2, 128).T
    bias = np.asarray(bias, dtype=np.float32)
    wconsts = np.zeros((128, 7), dtype=np.float32)
    wconsts[:, 0:2] = cw
    wconsts[:, 2:4] = qw
    wconsts[:, 4:6] = cqw
    wconsts[0, 6] = bias[0]
    in_maps = [
        {
            "ct": np.ascontiguousarray(c[b].T).astype(np.float16),
            "qt": np.ascontiguousarray(q[b].T).astype(np.float16),
            "wconsts": wconsts,
        }
        for b in range(B)
    ]
    res = run_bass_kernel_spmd(nc, in_maps, core_ids=list(range(B)), trace=TRACE)
    LAST_RESULTS = res
    return np.stack([res.results[b]["s"].astype(np.float32) for b in range(B)],
                    axis=0)
